# revision 9
# baseline (speedup 1.0000x reference)
"""DiversityLoss kernel for 8 Trainium2 NeuronCores.

Reference computes:
    loss = exp(mean(-D_img * D_noise))
where D_x[i,j] = (||x_i||^2 + ||x_j||^2 - 2 (X X^T)_ij) / d_x  for X in
{images, noises}.

The pairwise matrices never need to be materialized.  With
    a_i = ||img_i||^2, b_i = ||noise_i||^2, S1 = sum a, S2 = sum b,
    S3 = a.b, S4 = (Y^T a).(Y^T 1), S5 = (X^T b).(X^T 1), S6 = ||X^T Y||_F^2
the sum over all (i,j) of D_img*D_noise * (d_x*d_y) expands exactly to
    2*N*S3 + 2*S1*S2 - 4*S4 - 4*S5 + 4*S6.

Split of work:
  - S1..S5 are O(N*d) GEMV-scale terms -> computed exactly on the host in
    fp32/fp64 (the host already reads the full fp32 inputs to quantize).
    These are also the precision-critical terms.
  - S6 = ||X^T Y||_F^2 is the only O(N*dx*dy) term -> device.  Its share of
    the final sum is ~0.025%, so fp8 precision is ample (the fp8 square
    bias is corrected by the analytic constant C_SQ^2; E[fp8e4m3(x)^2] =
    C_SQ * E[x^2] for x ~ N(0,1)).

Sharding: the feature (column) axis of the flattened images is split across
the 8 cores (1536 columns each); noises Y is replicated.  S6 splits over
column chunks with no cross-core reduction.

Per-core device program: one input tensor in DoubleRow-interleaved layout
[128, 16 pairs, 2, 1792] fp8 where cols 0:256 of each pair-row are the Y
pair rows and cols 256:1792 the x pair rows.  The DMA streams pair chunks
in consumption order; for each pair the matmul makes the 128-col Y chunk
stationary (2 LDWEIGHTS per pair instead of 12 -- LDWEIGHTS at 1 row/cycle
@1.2GHz is the PE bottleneck otherwise) and streams x in 3 x 512 columns
into 6 PSUM banks (one accumulation group per bank, accumulated over all
16 pairs).  Drains: per-bank Z^2 partials, split ScalarE (activation
Square, single PSUM read) / VectorE (copy + fused mult+reduce).  PE HAM
warm-up dummies + an early activation-table preload keep the ramp off the
critical path.  Output: f [128, 8] f32 column partials of S6.
"""

import os
import sys

import numpy as np

for _p in ("/opt/trn_rl_repo", "/root/.axon_site/_ro/trn_rl_repo"):
    if os.path.isdir(_p) and _p not in sys.path:
        sys.path.append(_p)

import ml_dtypes

N = 4096
DX = 12288
DY = 256
NCORES = 8
KC = DX // NCORES        # 1536 x-columns per core
T = N // 128             # 32 row tiles of 128
Q = T // 2               # 16 DoubleRow pair-tiles
PCOLS = DY + KC          # combined [Y | x] row: 1792 fp8 per (pair, s)
CHUNK_PAIRS = (1, 1, 2, 2, 2, 2, 2, 2, 2)   # DMA chunking of the 16 pairs
NWARM = 12               # PE HAM warm-up dummy matmuls (must end before pair-0 data)

# E[fp8e4m3(x)^2] for x ~ N(0,1)  (exact; see module docstring)
C_SQ = 0.999275342216946

_PROG = None


def _build_program():
    from contextlib import ExitStack

    import concourse.bass as bass
    import concourse.tile as tile
    from concourse import bacc, mybir

    nc = bacc.Bacc(
        "TRN2",
        target_bir_lowering=False,
        debug=False,
        enable_asserts=False,
        num_devices=NCORES,
    )
    f32 = mybir.dt.float32
    bf16 = mybir.dt.bfloat16
    f8 = mybir.dt.float8e4
    DR = mybir.MatmulPerfMode.DoubleRow
    MULT = mybir.AluOpType.mult
    SQ = mybir.ActivationFunctionType.Square

    xy = nc.dram_tensor("xy", [128, Q, 2, PCOLS], f8, kind="ExternalInput").ap()
    f_out = nc.dram_tensor("f", [128, 8], f32, kind="ExternalOutput").ap()

    with tile.TileContext(nc) as tc, ExitStack() as ctx:
        data = ctx.enter_context(tc.tile_pool(name="data", bufs=1))
        scr = ctx.enter_context(tc.tile_pool(name="scr", bufs=2))
        stats = ctx.enter_context(tc.tile_pool(name="stats", bufs=1))
        zp = ctx.enter_context(tc.tile_pool(name="zp", bufs=1, space="PSUM"))

        s6 = stats.tile([128, 8], f32)
        dsrc = stats.tile([128, 128], f8)
        XY = data.tile([128, Q, 2, PCOLS], f8, name="XY")

        # All input DMAs on ONE ring in exact consumption order (strict FIFO
        # per ring -> the first chunk's descriptors drain first; a second
        # ring would round-robin SDMA packets and delay pair 0).  Pair 0 is
        # further split by columns so the first matmul (Y chunk + x cols
        # 0:512) starts after only 196KB.
        nc.sync.dma_start(XY[:, 0:1, :, 0:768], xy[:, 0:1, :, 0:768])
        nc.sync.dma_start(XY[:, 0:1, :, 768:1280], xy[:, 0:1, :, 768:1280])
        nc.sync.dma_start(XY[:, 0:1, :, 1280:PCOLS], xy[:, 0:1, :, 1280:PCOLS])
        q0 = 1
        for nq in CHUNK_PAIRS[1:]:
            nc.sync.dma_start(XY[:, q0 : q0 + nq, :, :], xy[:, q0 : q0 + nq, :, :])
            q0 += nq
        assert q0 == Q

        # warm-up while the first chunks stream: DVE memset feeds PE dummy
        # matmuls (HAM clock ramp) and ACT preloads the Square table.
        nc.vector.memset(dsrc[:], 0.0)
        nc.vector.memset(s6[:, 6:8], 0.0)
        tw = scr.tile([128, 1], bf16, tag="tw", name="tw")
        nc.scalar.activation(tw[:], dsrc[:, 0:1], SQ)
        zpad = zp.tile([128, 128], f32, name="zpad")
        for _ in range(NWARM):
            nc.tensor.matmul(zpad[:], lhsT=dsrc[:], rhs=dsrc[:], start=True, stop=True)

        # 6 accumulation groups g = yc*3 + xc, one PSUM bank each:
        # Z_g = Ychunk_yc^T @ Xchunk_xc accumulated over the 16 pairs.
        # Pair 0 runs xc-outer so each column piece unblocks two matmuls as
        # it lands; later pairs run yc-outer (2 LDWEIGHTS per pair).
        zb = [zp.tile([128, 512], f32, name=f"z{g}") for g in range(6)]
        for q in range(Q):
            if q == 0:
                order = [(yc, xc) for xc in range(3) for yc in range(2)]
            else:
                order = [(yc, xc) for yc in range(2) for xc in range(3)]
            for yc, xc in order:
                g = yc * 3 + xc
                nc.tensor.matmul(
                    zb[g][:],
                    lhsT=XY[:, q, :, 128 * yc : 128 * (yc + 1)],
                    rhs=XY[:, q, :, DY + 512 * xc : DY + 512 * (xc + 1)],
                    perf_mode=DR,
                    start=(q == 0),
                    stop=(q == Q - 1),
                )

        # drains: S6 partials per bank.  ACT is ~0.76us/bank (single PSUM
        # read activation Square), DVE ~1.25us/bank (copy out of PSUM, then
        # fused square+reduce from SBUF -- PSUM allows only one non-scalar
        # input), so split 4 ACT / 2 DVE, each chain in bank-stop order.
        for g in (0, 2, 4, 5):
            zsq = scr.tile([128, 512], bf16, tag="zsq", name=f"zsq{g}")
            nc.scalar.activation(zsq[:], zb[g][:], SQ, accum_out=s6[:, g : g + 1])
        for g in (1, 3):
            zcp = scr.tile([128, 512], f32, tag="zcp", name=f"zcp{g}")
            nc.vector.tensor_copy(zcp[:], zb[g][:])
            jnk = scr.tile([128, 512], bf16, tag="jnk", name=f"jnk{g}")
            nc.vector.scalar_tensor_tensor(
                out=jnk[:],
                in0=zcp[:],
                scalar=1.0,
                in1=zcp[:],
                op0=MULT,
                op1=MULT,
                accum_out=s6[:, g : g + 1],
            )

        # single contiguous [128 x 32B] output DMA (a column split would
        # make 16B strided HBM writes and pay the completion latency twice).
        nc.sync.dma_start(f_out, s6[:])

    nc.compile()
    return nc


def _get_program():
    global _PROG
    if _PROG is None:
        _PROG = _build_program()
    return _PROG


_LAST_RESULTS = None


def kernel(noises: np.ndarray, images: np.ndarray) -> np.ndarray:
    from concourse import bass_utils

    global _LAST_RESULTS

    nc = _get_program()

    X = np.ascontiguousarray(images, dtype=np.float32).reshape(N, -1)
    Y = np.ascontiguousarray(noises, dtype=np.float32)

    # exact host-side terms (S1..S5); f32 BLAS/einsum accumulation is ~1e-6
    # relative here, far inside the error budget.
    a = np.einsum("ij,ij->i", X, X).astype(np.float64)
    b = np.einsum("ij,ij->i", Y, Y).astype(np.float64)
    S1 = a.sum()
    S2 = b.sum()
    S3 = a @ b
    Y64 = Y.astype(np.float64)
    S4 = (Y64.T @ a) @ Y64.sum(axis=0)
    u = (X.T @ b.astype(np.float32)).astype(np.float64)
    sx = X.sum(axis=0).astype(np.float64)
    S5 = u @ sx

    # device inputs: fp8 DoubleRow-interleaved [128, Q, 2, 1792] with the Y
    # pair rows in cols 0:256 (replicated) and the core's x columns after.
    x8 = X.astype(ml_dtypes.float8_e4m3)
    y8 = Y.astype(ml_dtypes.float8_e4m3)
    y8i = np.ascontiguousarray(y8.reshape(Q, 2, 128, DY).transpose(2, 0, 1, 3))

    in_maps = []
    for c in range(NCORES):
        comb = np.empty((128, Q, 2, PCOLS), dtype=ml_dtypes.float8_e4m3)
        comb[..., :DY] = y8i
        comb[..., DY:] = (
            x8[:, c * KC : (c + 1) * KC].reshape(Q, 2, 128, KC).transpose(2, 0, 1, 3)
        )
        in_maps.append({"xy": comb})

    res = bass_utils.run_bass_kernel_spmd(nc, in_maps, core_ids=list(range(NCORES)))
    _LAST_RESULTS = res

    S6 = 0.0
    for c in range(NCORES):
        Fc = np.asarray(res.results[c]["f"], dtype=np.float64)
        S6 += Fc[:, 0:6].sum()
    S6 /= C_SQ * C_SQ

    num = 2.0 * N * S3 + 2.0 * S1 * S2 - 4.0 * S4 - 4.0 * S5 + 4.0 * S6
    mean = num / (float(N) * N * DX * DY)
    return np.asarray(np.exp(-mean), dtype=np.float32)


# revision 12
# speedup vs baseline: 1.1120x; 1.1120x over previous
"""DiversityLoss kernel for 8 Trainium2 NeuronCores.

Reference computes:
    loss = exp(mean(-D_img * D_noise))
where D_x[i,j] = (||x_i||^2 + ||x_j||^2 - 2 (X X^T)_ij) / d_x  for X in
{images, noises}.

The pairwise matrices never need to be materialized.  With
    a_i = ||img_i||^2, b_i = ||noise_i||^2, S1 = sum a, S2 = sum b,
    S3 = a.b, S4 = (Y^T a).(Y^T 1), S5 = (X^T b).(X^T 1), S6 = ||X^T Y||_F^2
the sum over all (i,j) of D_img*D_noise * (d_x*d_y) expands exactly to
    2*N*S3 + 2*S1*S2 - 4*S4 - 4*S5 + 4*S6.

Split of work:
  - S1..S5 are O(N*d) GEMV-scale terms -> computed exactly on the host in
    fp32/fp64 (the host already reads the full fp32 inputs to quantize).
    These are also the precision-critical terms.
  - S6 = ||X^T Y||_F^2 is the only O(N*dx*dy) term -> device.  Its share of
    the final sum is ~0.025%, so fp8 precision is ample (the fp8 square
    bias is corrected by the analytic constant C_SQ^2; E[fp8e4m3(x)^2] =
    C_SQ * E[x^2] for x ~ N(0,1)).

Sharding: the feature (column) axis of the flattened images is split across
the 8 cores (1536 columns each); noises Y is replicated.  S6 splits over
column chunks with no cross-core reduction.

Per-core device program: one input tensor in DoubleRow-interleaved layout
[128, 16 pairs, 2, 1792] fp8 where cols 0:256 of each pair-row are the Y
pair rows and cols 256:1792 the x pair rows.  The DMA streams pair chunks
in consumption order; for each pair the matmul makes the 128-col Y chunk
stationary (2 LDWEIGHTS per pair instead of 12 -- LDWEIGHTS at 1 row/cycle
@1.2GHz is the PE bottleneck otherwise) and streams x in 3 x 512 columns
into 6 PSUM banks (one accumulation group per bank, accumulated over all
16 pairs).  Drains: per-bank Z^2 partials, split ScalarE (activation
Square, single PSUM read) / VectorE (copy + fused mult+reduce).  PE HAM
warm-up dummies + an early activation-table preload keep the ramp off the
critical path.  Output: f [128, 8] f32 column partials of S6.
"""

import os
import sys

import numpy as np

for _p in ("/opt/trn_rl_repo", "/root/.axon_site/_ro/trn_rl_repo"):
    if os.path.isdir(_p) and _p not in sys.path:
        sys.path.append(_p)

import ml_dtypes

N = 4096
DX = 12288
DY = 256
NCORES = 8
KC = DX // NCORES        # 1536 x-columns per core
T = N // 128             # 32 row tiles of 128
Q = T // 2               # 16 DoubleRow pair-tiles
PCOLS = DY + KC          # combined [Y | x] row: 1792 fp8 per (pair, s)
CHUNK_PAIRS = (1, 1, 2, 2, 2, 2, 2, 2, 2)   # DMA chunking of the 16 pairs
NWARM = 34               # PE HAM warm-up dummy matmuls (bridge until pair-0 data)

# E[fp8e4m3(x)^2] for x ~ N(0,1)  (exact; see module docstring)
C_SQ = 0.999275342216946

_PROG = None


def _build_program():
    from contextlib import ExitStack

    import concourse.bass as bass
    import concourse.tile as tile
    from concourse import bacc, mybir

    nc = bacc.Bacc(
        "TRN2",
        target_bir_lowering=False,
        debug=False,
        enable_asserts=False,
        num_devices=NCORES,
    )
    f32 = mybir.dt.float32
    bf16 = mybir.dt.bfloat16
    f8 = mybir.dt.float8e4
    DR = mybir.MatmulPerfMode.DoubleRow
    MULT = mybir.AluOpType.mult
    SQ = mybir.ActivationFunctionType.Square

    xy = nc.dram_tensor("xy", [128, Q, 2, PCOLS], f8, kind="ExternalInput").ap()
    f_out = nc.dram_tensor("f", [128, 8], f32, kind="ExternalOutput").ap()

    with tile.TileContext(nc) as tc, ExitStack() as ctx:
        data = ctx.enter_context(tc.tile_pool(name="data", bufs=1))
        scr = ctx.enter_context(tc.tile_pool(name="scr", bufs=2))
        stats = ctx.enter_context(tc.tile_pool(name="stats", bufs=1))
        zp = ctx.enter_context(tc.tile_pool(name="zp", bufs=1, space="PSUM"))

        s6 = stats.tile([128, 8], f32)
        dsrc = stats.tile([128, 128], f8)
        XY = data.tile([128, Q, 2, PCOLS], f8, name="XY")

        # All input DMAs on ONE ring in exact consumption order (strict FIFO
        # per ring -> the first chunk's descriptors drain first; a second
        # ring would round-robin SDMA packets and delay pair 0).  Pair 0 is
        # further split by columns so the first matmul (Y chunk + x cols
        # 0:512) starts after only 196KB.
        for q in (0, 1):
            nc.sync.dma_start(XY[:, q : q + 1, :, 0:768], xy[:, q : q + 1, :, 0:768])
            nc.sync.dma_start(
                XY[:, q : q + 1, :, 768:1280], xy[:, q : q + 1, :, 768:1280]
            )
            nc.sync.dma_start(
                XY[:, q : q + 1, :, 1280:PCOLS], xy[:, q : q + 1, :, 1280:PCOLS]
            )
        q0 = 2
        for nq in CHUNK_PAIRS[2:]:
            nc.sync.dma_start(XY[:, q0 : q0 + nq, :, :], xy[:, q0 : q0 + nq, :, :])
            q0 += nq
        assert q0 == Q

        # warm-up while the first chunks stream: DVE memset feeds PE dummy
        # matmuls (HAM clock ramp) and ACT preloads the Square table.
        nc.vector.memset(dsrc[:], 0.0)
        nc.vector.memset(s6[:, 6:8], 0.0)
        tw = scr.tile([128, 1], bf16, tag="tw", name="tw")
        nc.scalar.activation(tw[:], dsrc[:, 0:1], SQ)
        zpad = zp.tile([128, 128], f32, name="zpad")
        for _ in range(NWARM):
            nc.tensor.matmul(zpad[:], lhsT=dsrc[:], rhs=dsrc[:], start=True, stop=True)

        # 6 accumulation groups g = yc*3 + xc, one PSUM bank each:
        # Z_g = Ychunk_yc^T @ Xchunk_xc accumulated over the 16 pairs.
        # Pair 0 runs xc-outer so each column piece unblocks two matmuls as
        # it lands; later pairs run yc-outer (2 LDWEIGHTS per pair).
        zb = [zp.tile([128, 512], f32, name=f"z{g}") for g in range(6)]
        for q in range(Q):
            if q <= 1:
                order = [(yc, xc) for xc in range(3) for yc in range(2)]
            else:
                order = [(yc, xc) for yc in range(2) for xc in range(3)]
            for yc, xc in order:
                g = yc * 3 + xc
                nc.tensor.matmul(
                    zb[g][:],
                    lhsT=XY[:, q, :, 128 * yc : 128 * (yc + 1)],
                    rhs=XY[:, q, :, DY + 512 * xc : DY + 512 * (xc + 1)],
                    perf_mode=DR,
                    start=(q == 0),
                    stop=(q == Q - 1),
                )

        # drains: S6 partials per bank.  ACT is ~0.76us/bank (single PSUM
        # read activation Square), DVE ~1.25us/bank (copy out of PSUM, then
        # fused square+reduce from SBUF -- PSUM allows only one non-scalar
        # input), so split 4 ACT / 2 DVE, each chain in bank-stop order.
        for g in (0, 2, 4, 5):
            zsq = scr.tile([128, 512], bf16, tag="zsq", name=f"zsq{g}")
            nc.scalar.activation(zsq[:], zb[g][:], SQ, accum_out=s6[:, g : g + 1])
        for g in (1, 3):
            zcp = scr.tile([128, 512], f32, tag="zcp", name=f"zcp{g}")
            nc.vector.tensor_copy(zcp[:], zb[g][:])
            jnk = scr.tile([128, 512], bf16, tag="jnk", name=f"jnk{g}")
            nc.vector.scalar_tensor_tensor(
                out=jnk[:],
                in0=zcp[:],
                scalar=1.0,
                in1=zcp[:],
                op0=MULT,
                op1=MULT,
                accum_out=s6[:, g : g + 1],
            )

        # single contiguous [128 x 32B] output DMA (a column split would
        # make 16B strided HBM writes and pay the completion latency twice).
        nc.sync.dma_start(f_out, s6[:])

    nc.compile()
    return nc


def _get_program():
    global _PROG
    if _PROG is None:
        _PROG = _build_program()
    return _PROG


_LAST_RESULTS = None


def kernel(noises: np.ndarray, images: np.ndarray) -> np.ndarray:
    from concourse import bass_utils

    global _LAST_RESULTS

    nc = _get_program()

    X = np.ascontiguousarray(images, dtype=np.float32).reshape(N, -1)
    Y = np.ascontiguousarray(noises, dtype=np.float32)

    # exact host-side terms (S1..S5); f32 BLAS/einsum accumulation is ~1e-6
    # relative here, far inside the error budget.
    a = np.einsum("ij,ij->i", X, X).astype(np.float64)
    b = np.einsum("ij,ij->i", Y, Y).astype(np.float64)
    S1 = a.sum()
    S2 = b.sum()
    S3 = a @ b
    Y64 = Y.astype(np.float64)
    S4 = (Y64.T @ a) @ Y64.sum(axis=0)
    u = (X.T @ b.astype(np.float32)).astype(np.float64)
    sx = X.sum(axis=0).astype(np.float64)
    S5 = u @ sx

    # device inputs: fp8 DoubleRow-interleaved [128, Q, 2, 1792] with the Y
    # pair rows in cols 0:256 (replicated) and the core's x columns after.
    x8 = X.astype(ml_dtypes.float8_e4m3)
    y8 = Y.astype(ml_dtypes.float8_e4m3)
    y8i = np.ascontiguousarray(y8.reshape(Q, 2, 128, DY).transpose(2, 0, 1, 3))

    in_maps = []
    for c in range(NCORES):
        comb = np.empty((128, Q, 2, PCOLS), dtype=ml_dtypes.float8_e4m3)
        comb[..., :DY] = y8i
        comb[..., DY:] = (
            x8[:, c * KC : (c + 1) * KC].reshape(Q, 2, 128, KC).transpose(2, 0, 1, 3)
        )
        in_maps.append({"xy": comb})

    res = bass_utils.run_bass_kernel_spmd(nc, in_maps, core_ids=list(range(NCORES)))
    _LAST_RESULTS = res

    S6 = 0.0
    for c in range(NCORES):
        Fc = np.asarray(res.results[c]["f"], dtype=np.float64)
        S6 += Fc[:, 0:6].sum()
    S6 /= C_SQ * C_SQ

    num = 2.0 * N * S3 + 2.0 * S1 * S2 - 4.0 * S4 - 4.0 * S5 + 4.0 * S6
    mean = num / (float(N) * N * DX * DY)
    return np.asarray(np.exp(-mean), dtype=np.float32)


# revision 21
# speedup vs baseline: 1.1347x; 1.0204x over previous
"""DiversityLoss kernel for 8 Trainium2 NeuronCores.

Reference computes:
    loss = exp(mean(-D_img * D_noise))
where D_x[i,j] = (||x_i||^2 + ||x_j||^2 - 2 (X X^T)_ij) / d_x  for X in
{images, noises}.

The pairwise matrices never need to be materialized.  With
    a_i = ||img_i||^2, b_i = ||noise_i||^2, S1 = sum a, S2 = sum b,
    S3 = a.b, S4 = (Y^T a).(Y^T 1), S5 = (X^T b).(X^T 1), S6 = ||X^T Y||_F^2
the sum over all (i,j) of D_img*D_noise * (d_x*d_y) expands exactly to
    2*N*S3 + 2*S1*S2 - 4*S4 - 4*S5 + 4*S6.

Split of work:
  - S1..S5 are O(N*d) GEMV-scale terms -> computed exactly on the host in
    fp32/fp64 (the host already reads the full fp32 inputs to quantize).
    These are also the precision-critical terms.
  - S6 = ||X^T Y||_F^2 is the only O(N*dx*dy) term -> device.  Its share of
    the final sum is ~0.025%, so fp8 precision is ample (the fp8 square
    bias is corrected by the analytic constant C_SQ^2; E[fp8e4m3(x)^2] =
    C_SQ * E[x^2] for x ~ N(0,1)).

Sharding: the feature (column) axis of the flattened images is split across
the 8 cores (1536 columns each); noises Y is replicated.  S6 splits over
column chunks with no cross-core reduction.

Per-core device program: one input tensor in DoubleRow-interleaved layout
[128, 16 pairs, 2, 1792] fp8 where cols 0:256 of each pair-row are the Y
pair rows and cols 256:1792 the x pair rows.  The DMA streams pair chunks
in consumption order; for each pair the matmul makes the 128-col Y chunk
stationary (2 LDWEIGHTS per pair instead of 12 -- LDWEIGHTS at 1 row/cycle
@1.2GHz is the PE bottleneck otherwise) and streams x in 3 x 512 columns
into 6 PSUM banks (one accumulation group per bank, accumulated over all
16 pairs).  Drains: per-bank Z^2 partials, split ScalarE (activation
Square, single PSUM read) / VectorE (copy + fused mult+reduce).  PE HAM
warm-up dummies + an early activation-table preload keep the ramp off the
critical path.  Output: f [128, 8] f32 column partials of S6.
"""

import os
import sys

import numpy as np

for _p in ("/opt/trn_rl_repo", "/root/.axon_site/_ro/trn_rl_repo"):
    if os.path.isdir(_p) and _p not in sys.path:
        sys.path.append(_p)

import ml_dtypes

N = 4096
DX = 12288
DY = 256
NCORES = 8
KC = DX // NCORES        # 1536 x-columns per core
T = N // 128             # 32 row tiles of 128
Q = T // 2               # 16 DoubleRow pair-tiles
PCOLS = DY + KC          # combined [Y | x] row: 1792 fp8 per (pair, s)
CHUNK_PAIRS = (1, 1, 2, 2, 2, 2, 2, 2, 1, 1)   # DMA chunking of the 16 pairs
NWARM = 40               # PE HAM warm-up dummy matmuls (bridge until pair-0 data)

# E[fp8e4m3(x)^2] for x ~ N(0,1)  (exact; see module docstring)
C_SQ = 0.999275342216946

_PROG = None


def _build_program():
    from contextlib import ExitStack

    import concourse.bass as bass
    import concourse.tile as tile
    from concourse import bacc, mybir

    nc = bacc.Bacc(
        "TRN2",
        target_bir_lowering=False,
        debug=False,
        enable_asserts=False,
        num_devices=NCORES,
    )
    f32 = mybir.dt.float32
    bf16 = mybir.dt.bfloat16
    f8 = mybir.dt.float8e4
    DR = mybir.MatmulPerfMode.DoubleRow
    MULT = mybir.AluOpType.mult
    SQ = mybir.ActivationFunctionType.Square

    xy = nc.dram_tensor("xy", [128, Q, 2, PCOLS], f8, kind="ExternalInput").ap()
    f_out = nc.dram_tensor("f", [128, 6], f32, kind="ExternalOutput").ap()

    with tile.TileContext(nc) as tc, ExitStack() as ctx:
        data = ctx.enter_context(tc.tile_pool(name="data", bufs=1))
        scr = ctx.enter_context(tc.tile_pool(name="scr", bufs=2))
        stats = ctx.enter_context(tc.tile_pool(name="stats", bufs=1))
        zp = ctx.enter_context(tc.tile_pool(name="zp", bufs=1, space="PSUM"))

        s6 = stats.tile([128, 6], f32)
        dsrc = stats.tile([128, 128], f8)
        XY = data.tile([128, Q, 2, PCOLS], f8, name="XY")

        # All input DMAs on ONE ring in exact consumption order (strict FIFO
        # per ring -> the first chunk's descriptors drain first; a second
        # ring would round-robin SDMA packets and delay pair 0).  Pair 0 is
        # further split by columns so the first matmul (Y chunk + x cols
        # 0:512) starts after only 196KB.
        nc.sync.dma_start(XY[:, 0:1, :, 0:768], xy[:, 0:1, :, 0:768])
        nc.sync.dma_start(XY[:, 0:1, :, 768:1280], xy[:, 0:1, :, 768:1280])
        nc.sync.dma_start(XY[:, 0:1, :, 1280:PCOLS], xy[:, 0:1, :, 1280:PCOLS])
        q0 = 1
        for nq in CHUNK_PAIRS[1:]:
            nc.sync.dma_start(XY[:, q0 : q0 + nq, :, :], xy[:, q0 : q0 + nq, :, :])
            q0 += nq
        assert q0 == Q

        # warm-up while the first chunks stream: DVE memset feeds PE dummy
        # matmuls (HAM clock ramp) and ACT preloads the Square table.
        nc.vector.memset(dsrc[:], 0.0)
        tw = scr.tile([128, 1], bf16, tag="tw", name="tw")
        nc.scalar.activation(tw[:], dsrc[:, 0:1], SQ)
        zpad = zp.tile([128, 128], f32, name="zpad")
        for _ in range(NWARM):
            nc.tensor.matmul(zpad[:], lhsT=dsrc[:], rhs=dsrc[:], start=True, stop=True)

        # 6 accumulation groups g = yc*3 + xc, one PSUM bank each:
        # Z_g = Ychunk_yc^T @ Xchunk_xc accumulated over the 16 pairs.
        # Pair 0 runs xc-outer so each column piece unblocks two matmuls as
        # it lands; later pairs run yc-outer (2 LDWEIGHTS per pair).
        zb = [zp.tile([128, 512], f32, name=f"z{g}") for g in range(6)]
        for q in range(Q):
            if q == 0:
                order = [(yc, xc) for xc in range(3) for yc in range(2)]
            else:
                order = [(yc, xc) for yc in range(2) for xc in range(3)]
            for yc, xc in order:
                g = yc * 3 + xc
                nc.tensor.matmul(
                    zb[g][:],
                    lhsT=XY[:, q, :, 128 * yc : 128 * (yc + 1)],
                    rhs=XY[:, q, :, DY + 512 * xc : DY + 512 * (xc + 1)],
                    perf_mode=DR,
                    start=(q == 0),
                    stop=(q == Q - 1),
                )

        # drains: S6 partials per bank.  ACT is ~0.76us/bank (single PSUM
        # read activation Square), DVE ~1.25us/bank (copy out of PSUM, then
        # fused square+reduce from SBUF -- PSUM allows only one non-scalar
        # input), so split 4 ACT / 2 DVE, each chain in bank-stop order.
        for g in (0, 2, 4, 5):
            zsq = scr.tile([128, 512], bf16, tag="zsq", name=f"zsq{g}")
            nc.scalar.activation(zsq[:], zb[g][:], SQ, accum_out=s6[:, g : g + 1])
        for g in (1, 3):
            zcp = scr.tile([128, 512], f32, tag="zcp", name=f"zcp{g}")
            nc.vector.tensor_copy(zcp[:], zb[g][:])
            jnk = scr.tile([128, 512], bf16, tag="jnk", name=f"jnk{g}")
            nc.vector.scalar_tensor_tensor(
                out=jnk[:],
                in0=zcp[:],
                scalar=1.0,
                in1=zcp[:],
                op0=MULT,
                op1=MULT,
                accum_out=s6[:, g : g + 1],
            )

        # single contiguous output DMA on the otherwise-idle scalar ring (a
        # column split would make strided HBM writes and pay the completion
        # latency twice).
        nc.scalar.dma_start(f_out, s6[:])

    nc.compile()
    return nc


def _get_program():
    global _PROG
    if _PROG is None:
        _PROG = _build_program()
    return _PROG


_LAST_RESULTS = None


def kernel(noises: np.ndarray, images: np.ndarray) -> np.ndarray:
    from concourse import bass_utils

    global _LAST_RESULTS

    nc = _get_program()

    X = np.ascontiguousarray(images, dtype=np.float32).reshape(N, -1)
    Y = np.ascontiguousarray(noises, dtype=np.float32)

    # exact host-side terms (S1..S5); f32 BLAS/einsum accumulation is ~1e-6
    # relative here, far inside the error budget.
    a = np.einsum("ij,ij->i", X, X).astype(np.float64)
    b = np.einsum("ij,ij->i", Y, Y).astype(np.float64)
    S1 = a.sum()
    S2 = b.sum()
    S3 = a @ b
    Y64 = Y.astype(np.float64)
    S4 = (Y64.T @ a) @ Y64.sum(axis=0)
    u = (X.T @ b.astype(np.float32)).astype(np.float64)
    sx = X.sum(axis=0).astype(np.float64)
    S5 = u @ sx

    # device inputs: fp8 DoubleRow-interleaved [128, Q, 2, 1792] with the Y
    # pair rows in cols 0:256 (replicated) and the core's x columns after.
    x8 = X.astype(ml_dtypes.float8_e4m3)
    y8 = Y.astype(ml_dtypes.float8_e4m3)
    y8i = np.ascontiguousarray(y8.reshape(Q, 2, 128, DY).transpose(2, 0, 1, 3))

    in_maps = []
    for c in range(NCORES):
        comb = np.empty((128, Q, 2, PCOLS), dtype=ml_dtypes.float8_e4m3)
        comb[..., :DY] = y8i
        comb[..., DY:] = (
            x8[:, c * KC : (c + 1) * KC].reshape(Q, 2, 128, KC).transpose(2, 0, 1, 3)
        )
        in_maps.append({"xy": comb})

    res = bass_utils.run_bass_kernel_spmd(nc, in_maps, core_ids=list(range(NCORES)))
    _LAST_RESULTS = res

    S6 = 0.0
    for c in range(NCORES):
        Fc = np.asarray(res.results[c]["f"], dtype=np.float64)
        S6 += Fc.sum()
    S6 /= C_SQ * C_SQ

    num = 2.0 * N * S3 + 2.0 * S1 * S2 - 4.0 * S4 - 4.0 * S5 + 4.0 * S6
    mean = num / (float(N) * N * DX * DY)
    return np.asarray(np.exp(-mean), dtype=np.float32)


# revision 22
# speedup vs baseline: 1.1787x; 1.0388x over previous
"""DiversityLoss kernel for 8 Trainium2 NeuronCores.

Reference computes:
    loss = exp(mean(-D_img * D_noise))
where D_x[i,j] = (||x_i||^2 + ||x_j||^2 - 2 (X X^T)_ij) / d_x  for X in
{images, noises}.

The pairwise matrices never need to be materialized.  With
    a_i = ||img_i||^2, b_i = ||noise_i||^2, S1 = sum a, S2 = sum b,
    S3 = a.b, S4 = (Y^T a).(Y^T 1), S5 = (X^T b).(X^T 1), S6 = ||X^T Y||_F^2
the sum over all (i,j) of D_img*D_noise * (d_x*d_y) expands exactly to
    2*N*S3 + 2*S1*S2 - 4*S4 - 4*S5 + 4*S6.

Split of work:
  - S1..S5 are O(N*d) GEMV-scale terms -> computed exactly on the host in
    fp32/fp64 (the host already reads the full fp32 inputs to quantize).
    These are also the precision-critical terms.
  - S6 = ||X^T Y||_F^2 is the only O(N*dx*dy) term -> device.  Its share of
    the final sum is ~0.025%, so fp8 precision is ample (the fp8 square
    bias is corrected by the analytic constant C_SQ^2; E[fp8e4m3(x)^2] =
    C_SQ * E[x^2] for x ~ N(0,1)).

Sharding: the feature (column) axis of the flattened images is split across
the 8 cores (1536 columns each); noises Y is replicated.  S6 splits over
column chunks with no cross-core reduction.

Per-core device program: one input tensor in DoubleRow-interleaved layout
[128, 16 pairs, 2, 1792] fp8 where cols 0:256 of each pair-row are the Y
pair rows and cols 256:1792 the x pair rows.  The DMA streams pair chunks
in consumption order; for each pair the matmul makes the 128-col Y chunk
stationary (2 LDWEIGHTS per pair instead of 12 -- LDWEIGHTS at 1 row/cycle
@1.2GHz is the PE bottleneck otherwise) and streams x in 3 x 512 columns
into 6 PSUM banks (one accumulation group per bank, accumulated over all
16 pairs).  Drains: per-bank Z^2 partials, split ScalarE (activation
Square, single PSUM read) / VectorE (copy + fused mult+reduce).  PE HAM
warm-up dummies + an early activation-table preload keep the ramp off the
critical path.  Output: f [128, 8] f32 column partials of S6.
"""

import os
import sys

import numpy as np

for _p in ("/opt/trn_rl_repo", "/root/.axon_site/_ro/trn_rl_repo"):
    if os.path.isdir(_p) and _p not in sys.path:
        sys.path.append(_p)

import ml_dtypes

N = 4096
DX = 12288
DY = 256
NCORES = 8
KC = DX // NCORES        # 1536 x-columns per core
T = N // 128             # 32 row tiles of 128
Q = T // 2               # 16 DoubleRow pair-tiles
PCOLS = DY + KC          # combined [Y | x] row: 1792 fp8 per (pair, s)
CHUNK_PAIRS = (1, 1, 2, 2, 2, 2, 2, 2, 1, 1)   # DMA chunking of the 16 pairs
NWARM = 40               # PE HAM warm-up dummy matmuls (bridge until pair-0 data)

# E[fp8e4m3(x)^2] for x ~ N(0,1)  (exact; see module docstring)
C_SQ = 0.999275342216946

_PROG = None


def _build_program():
    from contextlib import ExitStack

    import concourse.bass as bass
    import concourse.tile as tile
    from concourse import bacc, mybir

    nc = bacc.Bacc(
        "TRN2",
        target_bir_lowering=False,
        debug=False,
        enable_asserts=False,
        num_devices=NCORES,
    )
    f32 = mybir.dt.float32
    bf16 = mybir.dt.bfloat16
    f8 = mybir.dt.float8e4
    DR = mybir.MatmulPerfMode.DoubleRow
    MULT = mybir.AluOpType.mult
    SQ = mybir.ActivationFunctionType.Square

    xy = nc.dram_tensor("xy", [128, Q, 2, PCOLS], f8, kind="ExternalInput").ap()
    f_out = nc.dram_tensor("f", [128, 6], f32, kind="ExternalOutput").ap()

    with tile.TileContext(nc) as tc, ExitStack() as ctx:
        data = ctx.enter_context(tc.tile_pool(name="data", bufs=1))
        scr = ctx.enter_context(tc.tile_pool(name="scr", bufs=2))
        stats = ctx.enter_context(tc.tile_pool(name="stats", bufs=1))
        zp = ctx.enter_context(tc.tile_pool(name="zp", bufs=1, space="PSUM"))

        s6 = stats.tile([128, 6], f32)
        dsrc = stats.tile([128, 128], f8)
        XY = data.tile([128, Q, 2, PCOLS], f8, name="XY")

        # All input DMAs on ONE ring in exact consumption order (strict FIFO
        # per ring -> the first chunk's descriptors drain first; a second
        # ring would round-robin SDMA packets and delay pair 0).  Pair 0 is
        # further split by columns so the first matmul (Y chunk + x cols
        # 0:512) starts after only 196KB.
        nc.sync.dma_start(XY[:, 0:1, :, 0:768], xy[:, 0:1, :, 0:768])
        nc.sync.dma_start(XY[:, 0:1, :, 768:1280], xy[:, 0:1, :, 768:1280])
        nc.sync.dma_start(XY[:, 0:1, :, 1280:PCOLS], xy[:, 0:1, :, 1280:PCOLS])
        q0 = 1
        for nq in CHUNK_PAIRS[1:]:
            nc.sync.dma_start(XY[:, q0 : q0 + nq, :, :], xy[:, q0 : q0 + nq, :, :])
            q0 += nq
        assert q0 == Q

        # warm-up while the first chunks stream: DVE memset feeds PE dummy
        # matmuls (HAM clock ramp) and ACT preloads the Square table.
        nc.vector.memset(dsrc[:], 0.0)
        tw = scr.tile([128, 1], bf16, tag="tw", name="tw")
        nc.scalar.activation(tw[:], dsrc[:, 0:1], SQ)
        zpad = zp.tile([128, 128], f32, name="zpad")
        for _ in range(NWARM):
            nc.tensor.matmul(zpad[:], lhsT=dsrc[:], rhs=dsrc[:], start=True, stop=True)

        # 6 accumulation groups g = yc*3 + xc, one PSUM bank each:
        # Z_g = Ychunk_yc^T @ Xchunk_xc accumulated over the 16 pairs.
        # Pair 0 runs xc-outer so each column piece unblocks two matmuls as
        # it lands; later pairs run yc-outer (2 LDWEIGHTS per pair).
        zb = [zp.tile([128, 512], f32, name=f"z{g}") for g in range(6)]
        for q in range(Q):
            if q == 0:
                order = [(yc, xc) for xc in range(3) for yc in range(2)]
            else:
                order = [(yc, xc) for yc in range(2) for xc in range(3)]
            for yc, xc in order:
                g = yc * 3 + xc
                nc.tensor.matmul(
                    zb[g][:],
                    lhsT=XY[:, q, :, 128 * yc : 128 * (yc + 1)],
                    rhs=XY[:, q, :, DY + 512 * xc : DY + 512 * (xc + 1)],
                    perf_mode=DR,
                    start=(q == 0),
                    stop=(q == Q - 1),
                )

        # drains: S6 partials per bank.  ACT is ~0.76us/bank (single PSUM
        # read activation Square), DVE ~1.25us/bank (copy out of PSUM, then
        # fused square+reduce from SBUF -- PSUM allows only one non-scalar
        # input), so split 4 ACT / 2 DVE, each chain in bank-stop order.
        for g in (0, 2, 4, 5):
            zsq = scr.tile([128, 512], bf16, tag="zsq", name=f"zsq{g}")
            nc.scalar.activation(zsq[:], zb[g][:], SQ, accum_out=s6[:, g : g + 1])
        for g in (1, 3):
            zcp = scr.tile([128, 512], f32, tag="zcp", name=f"zcp{g}")
            nc.vector.tensor_copy(zcp[:], zb[g][:])
            jnk = scr.tile([128, 512], bf16, tag="jnk", name=f"jnk{g}")
            nc.vector.scalar_tensor_tensor(
                out=jnk[:],
                in0=zcp[:],
                scalar=1.0,
                in1=zcp[:],
                op0=MULT,
                op1=MULT,
                accum_out=s6[:, g : g + 1],
            )

        # single contiguous output DMA on the sync ring -- its HWDGE path is
        # warm from the input stream (the idle scalar ring measured ~1.5us
        # slower).  A column split would make strided HBM writes and pay the
        # completion latency twice.
        nc.sync.dma_start(f_out, s6[:])

    nc.compile()
    return nc


def _get_program():
    global _PROG
    if _PROG is None:
        _PROG = _build_program()
    return _PROG


_LAST_RESULTS = None


def kernel(noises: np.ndarray, images: np.ndarray) -> np.ndarray:
    from concourse import bass_utils

    global _LAST_RESULTS

    nc = _get_program()

    X = np.ascontiguousarray(images, dtype=np.float32).reshape(N, -1)
    Y = np.ascontiguousarray(noises, dtype=np.float32)

    # exact host-side terms (S1..S5); f32 BLAS/einsum accumulation is ~1e-6
    # relative here, far inside the error budget.
    a = np.einsum("ij,ij->i", X, X).astype(np.float64)
    b = np.einsum("ij,ij->i", Y, Y).astype(np.float64)
    S1 = a.sum()
    S2 = b.sum()
    S3 = a @ b
    Y64 = Y.astype(np.float64)
    S4 = (Y64.T @ a) @ Y64.sum(axis=0)
    u = (X.T @ b.astype(np.float32)).astype(np.float64)
    sx = X.sum(axis=0).astype(np.float64)
    S5 = u @ sx

    # device inputs: fp8 DoubleRow-interleaved [128, Q, 2, 1792] with the Y
    # pair rows in cols 0:256 (replicated) and the core's x columns after.
    x8 = X.astype(ml_dtypes.float8_e4m3)
    y8 = Y.astype(ml_dtypes.float8_e4m3)
    y8i = np.ascontiguousarray(y8.reshape(Q, 2, 128, DY).transpose(2, 0, 1, 3))

    in_maps = []
    for c in range(NCORES):
        comb = np.empty((128, Q, 2, PCOLS), dtype=ml_dtypes.float8_e4m3)
        comb[..., :DY] = y8i
        comb[..., DY:] = (
            x8[:, c * KC : (c + 1) * KC].reshape(Q, 2, 128, KC).transpose(2, 0, 1, 3)
        )
        in_maps.append({"xy": comb})

    res = bass_utils.run_bass_kernel_spmd(nc, in_maps, core_ids=list(range(NCORES)))
    _LAST_RESULTS = res

    S6 = 0.0
    for c in range(NCORES):
        Fc = np.asarray(res.results[c]["f"], dtype=np.float64)
        S6 += Fc.sum()
    S6 /= C_SQ * C_SQ

    num = 2.0 * N * S3 + 2.0 * S1 * S2 - 4.0 * S4 - 4.0 * S5 + 4.0 * S6
    mean = num / (float(N) * N * DX * DY)
    return np.asarray(np.exp(-mean), dtype=np.float32)


# revision 23
# speedup vs baseline: 1.1938x; 1.0128x over previous
"""DiversityLoss kernel for 8 Trainium2 NeuronCores.

Reference computes:
    loss = exp(mean(-D_img * D_noise))
where D_x[i,j] = (||x_i||^2 + ||x_j||^2 - 2 (X X^T)_ij) / d_x  for X in
{images, noises}.

The pairwise matrices never need to be materialized.  With
    a_i = ||img_i||^2, b_i = ||noise_i||^2, S1 = sum a, S2 = sum b,
    S3 = a.b, S4 = (Y^T a).(Y^T 1), S5 = (X^T b).(X^T 1), S6 = ||X^T Y||_F^2
the sum over all (i,j) of D_img*D_noise * (d_x*d_y) expands exactly to
    2*N*S3 + 2*S1*S2 - 4*S4 - 4*S5 + 4*S6.

Split of work:
  - S1..S5 are O(N*d) GEMV-scale terms -> computed exactly on the host in
    fp32/fp64 (the host already reads the full fp32 inputs to quantize).
    These are also the precision-critical terms.
  - S6 = ||X^T Y||_F^2 is the only O(N*dx*dy) term -> device.  Its share of
    the final sum is ~0.025%, so fp8 precision is ample (the fp8 square
    bias is corrected by the analytic constant C_SQ^2; E[fp8e4m3(x)^2] =
    C_SQ * E[x^2] for x ~ N(0,1)).

Sharding: the feature (column) axis of the flattened images is split across
the 8 cores (1536 columns each); noises Y is replicated.  S6 splits over
column chunks with no cross-core reduction.

Per-core device program: one input tensor in DoubleRow-interleaved layout
[128, 16 pairs, 2, 1792] fp8 where cols 0:256 of each pair-row are the Y
pair rows and cols 256:1792 the x pair rows.  The DMA streams pair chunks
in consumption order; for each pair the matmul makes the 128-col Y chunk
stationary (2 LDWEIGHTS per pair instead of 12 -- LDWEIGHTS at 1 row/cycle
@1.2GHz is the PE bottleneck otherwise) and streams x in 3 x 512 columns
into 6 PSUM banks (one accumulation group per bank, accumulated over all
16 pairs).  Drains: per-bank Z^2 partials, split ScalarE (activation
Square, single PSUM read) / VectorE (copy + fused mult+reduce).  PE HAM
warm-up dummies + an early activation-table preload keep the ramp off the
critical path.  Output: f [128, 8] f32 column partials of S6.
"""

import os
import sys

import numpy as np

for _p in ("/opt/trn_rl_repo", "/root/.axon_site/_ro/trn_rl_repo"):
    if os.path.isdir(_p) and _p not in sys.path:
        sys.path.append(_p)

import ml_dtypes

N = 4096
DX = 12288
DY = 256
NCORES = 8
KC = DX // NCORES        # 1536 x-columns per core
T = N // 128             # 32 row tiles of 128
Q = T // 2               # 16 DoubleRow pair-tiles
PCOLS = DY + KC          # combined [Y | x] row: 1792 fp8 per (pair, s)
CHUNK_PAIRS = (1, 1, 1, 1, 2, 2, 2, 2, 2, 1, 1)   # DMA chunking of the 16 pairs
NWARM = 40               # PE HAM warm-up dummy matmuls (bridge until pair-0 data)

# E[fp8e4m3(x)^2] for x ~ N(0,1)  (exact; see module docstring)
C_SQ = 0.999275342216946

_PROG = None


def _build_program():
    from contextlib import ExitStack

    import concourse.bass as bass
    import concourse.tile as tile
    from concourse import bacc, mybir

    nc = bacc.Bacc(
        "TRN2",
        target_bir_lowering=False,
        debug=False,
        enable_asserts=False,
        num_devices=NCORES,
    )
    f32 = mybir.dt.float32
    bf16 = mybir.dt.bfloat16
    f8 = mybir.dt.float8e4
    DR = mybir.MatmulPerfMode.DoubleRow
    MULT = mybir.AluOpType.mult
    SQ = mybir.ActivationFunctionType.Square

    xy = nc.dram_tensor("xy", [128, Q, 2, PCOLS], f8, kind="ExternalInput").ap()
    f_out = nc.dram_tensor("f", [128, 6], f32, kind="ExternalOutput").ap()

    with tile.TileContext(nc) as tc, ExitStack() as ctx:
        data = ctx.enter_context(tc.tile_pool(name="data", bufs=1))
        scr = ctx.enter_context(tc.tile_pool(name="scr", bufs=2))
        stats = ctx.enter_context(tc.tile_pool(name="stats", bufs=1))
        zp = ctx.enter_context(tc.tile_pool(name="zp", bufs=1, space="PSUM"))

        s6 = stats.tile([128, 6], f32)
        dsrc = stats.tile([128, 128], f8)
        XY = data.tile([128, Q, 2, PCOLS], f8, name="XY")

        # All input DMAs on ONE ring in exact consumption order (strict FIFO
        # per ring -> the first chunk's descriptors drain first; a second
        # ring would round-robin SDMA packets and delay pair 0).  Pair 0 is
        # further split by columns so the first matmul (Y chunk + x cols
        # 0:512) starts after only 196KB.
        nc.sync.dma_start(XY[:, 0:1, :, 0:768], xy[:, 0:1, :, 0:768])
        nc.sync.dma_start(XY[:, 0:1, :, 768:1280], xy[:, 0:1, :, 768:1280])
        nc.sync.dma_start(XY[:, 0:1, :, 1280:PCOLS], xy[:, 0:1, :, 1280:PCOLS])
        q0 = 1
        for nq in CHUNK_PAIRS[1:]:
            nc.sync.dma_start(XY[:, q0 : q0 + nq, :, :], xy[:, q0 : q0 + nq, :, :])
            q0 += nq
        assert q0 == Q

        # warm-up while the first chunks stream: DVE memset feeds PE dummy
        # matmuls (HAM clock ramp) and ACT preloads the Square table.
        nc.vector.memset(dsrc[:], 0.0)
        tw = scr.tile([128, 1], bf16, tag="tw", name="tw")
        nc.scalar.activation(tw[:], dsrc[:, 0:1], SQ)
        zpad = zp.tile([128, 128], f32, name="zpad")
        for _ in range(NWARM):
            nc.tensor.matmul(zpad[:], lhsT=dsrc[:], rhs=dsrc[:], start=True, stop=True)

        # 6 accumulation groups g = yc*3 + xc, one PSUM bank each:
        # Z_g = Ychunk_yc^T @ Xchunk_xc accumulated over the 16 pairs.
        # Pair 0 runs xc-outer so each column piece unblocks two matmuls as
        # it lands; later pairs run yc-outer (2 LDWEIGHTS per pair).
        zb = [zp.tile([128, 512], f32, name=f"z{g}") for g in range(6)]
        for q in range(Q):
            if q == 0:
                order = [(yc, xc) for xc in range(3) for yc in range(2)]
            else:
                order = [(yc, xc) for yc in range(2) for xc in range(3)]
            for yc, xc in order:
                g = yc * 3 + xc
                nc.tensor.matmul(
                    zb[g][:],
                    lhsT=XY[:, q, :, 128 * yc : 128 * (yc + 1)],
                    rhs=XY[:, q, :, DY + 512 * xc : DY + 512 * (xc + 1)],
                    perf_mode=DR,
                    start=(q == 0),
                    stop=(q == Q - 1),
                )

        # drains: S6 partials per bank.  ACT is ~0.76us/bank (single PSUM
        # read activation Square), DVE ~1.25us/bank (copy out of PSUM, then
        # fused square+reduce from SBUF -- PSUM allows only one non-scalar
        # input), so split 4 ACT / 2 DVE, each chain in bank-stop order.
        for g in (0, 2, 4, 5):
            zsq = scr.tile([128, 512], bf16, tag="zsq", name=f"zsq{g}")
            nc.scalar.activation(zsq[:], zb[g][:], SQ, accum_out=s6[:, g : g + 1])
        for g in (1, 3):
            zcp = scr.tile([128, 512], f32, tag="zcp", name=f"zcp{g}")
            nc.vector.tensor_copy(zcp[:], zb[g][:])
            jnk = scr.tile([128, 512], bf16, tag="jnk", name=f"jnk{g}")
            nc.vector.scalar_tensor_tensor(
                out=jnk[:],
                in0=zcp[:],
                scalar=1.0,
                in1=zcp[:],
                op0=MULT,
                op1=MULT,
                accum_out=s6[:, g : g + 1],
            )

        # single contiguous output DMA on the sync ring -- its HWDGE path is
        # warm from the input stream (the idle scalar ring measured ~1.5us
        # slower).  A column split would make strided HBM writes and pay the
        # completion latency twice.
        nc.sync.dma_start(f_out, s6[:])

    nc.compile()
    return nc


def _get_program():
    global _PROG
    if _PROG is None:
        _PROG = _build_program()
    return _PROG


_LAST_RESULTS = None


def kernel(noises: np.ndarray, images: np.ndarray) -> np.ndarray:
    from concourse import bass_utils

    global _LAST_RESULTS

    nc = _get_program()

    X = np.ascontiguousarray(images, dtype=np.float32).reshape(N, -1)
    Y = np.ascontiguousarray(noises, dtype=np.float32)

    # exact host-side terms (S1..S5); f32 BLAS/einsum accumulation is ~1e-6
    # relative here, far inside the error budget.
    a = np.einsum("ij,ij->i", X, X).astype(np.float64)
    b = np.einsum("ij,ij->i", Y, Y).astype(np.float64)
    S1 = a.sum()
    S2 = b.sum()
    S3 = a @ b
    Y64 = Y.astype(np.float64)
    S4 = (Y64.T @ a) @ Y64.sum(axis=0)
    u = (X.T @ b.astype(np.float32)).astype(np.float64)
    sx = X.sum(axis=0).astype(np.float64)
    S5 = u @ sx

    # device inputs: fp8 DoubleRow-interleaved [128, Q, 2, 1792] with the Y
    # pair rows in cols 0:256 (replicated) and the core's x columns after.
    x8 = X.astype(ml_dtypes.float8_e4m3)
    y8 = Y.astype(ml_dtypes.float8_e4m3)
    y8i = np.ascontiguousarray(y8.reshape(Q, 2, 128, DY).transpose(2, 0, 1, 3))

    in_maps = []
    for c in range(NCORES):
        comb = np.empty((128, Q, 2, PCOLS), dtype=ml_dtypes.float8_e4m3)
        comb[..., :DY] = y8i
        comb[..., DY:] = (
            x8[:, c * KC : (c + 1) * KC].reshape(Q, 2, 128, KC).transpose(2, 0, 1, 3)
        )
        in_maps.append({"xy": comb})

    res = bass_utils.run_bass_kernel_spmd(nc, in_maps, core_ids=list(range(NCORES)))
    _LAST_RESULTS = res

    S6 = 0.0
    for c in range(NCORES):
        Fc = np.asarray(res.results[c]["f"], dtype=np.float64)
        S6 += Fc.sum()
    S6 /= C_SQ * C_SQ

    num = 2.0 * N * S3 + 2.0 * S1 * S2 - 4.0 * S4 - 4.0 * S5 + 4.0 * S6
    mean = num / (float(N) * N * DX * DY)
    return np.asarray(np.exp(-mean), dtype=np.float32)


# revision 24
# speedup vs baseline: 1.2156x; 1.0183x over previous
"""DiversityLoss kernel for 8 Trainium2 NeuronCores.

Reference computes:
    loss = exp(mean(-D_img * D_noise))
where D_x[i,j] = (||x_i||^2 + ||x_j||^2 - 2 (X X^T)_ij) / d_x  for X in
{images, noises}.

The pairwise matrices never need to be materialized.  With
    a_i = ||img_i||^2, b_i = ||noise_i||^2, S1 = sum a, S2 = sum b,
    S3 = a.b, S4 = (Y^T a).(Y^T 1), S5 = (X^T b).(X^T 1), S6 = ||X^T Y||_F^2
the sum over all (i,j) of D_img*D_noise * (d_x*d_y) expands exactly to
    2*N*S3 + 2*S1*S2 - 4*S4 - 4*S5 + 4*S6.

Split of work:
  - S1..S5 are O(N*d) GEMV-scale terms -> computed exactly on the host in
    fp32/fp64 (the host already reads the full fp32 inputs to quantize).
    These are also the precision-critical terms.
  - S6 = ||X^T Y||_F^2 is the only O(N*dx*dy) term -> device.  Its share of
    the final sum is ~0.025%, so fp8 precision is ample (the fp8 square
    bias is corrected by the analytic constant C_SQ^2; E[fp8e4m3(x)^2] =
    C_SQ * E[x^2] for x ~ N(0,1)).

Sharding: the feature (column) axis of the flattened images is split across
the 8 cores (1536 columns each); noises Y is replicated.  S6 splits over
column chunks with no cross-core reduction.

Per-core device program: one input tensor in DoubleRow-interleaved layout
[128, 16 pairs, 2, 1792] fp8 where cols 0:256 of each pair-row are the Y
pair rows and cols 256:1792 the x pair rows.  The DMA streams pair chunks
in consumption order; for each pair the matmul makes the 128-col Y chunk
stationary (2 LDWEIGHTS per pair instead of 12 -- LDWEIGHTS at 1 row/cycle
@1.2GHz is the PE bottleneck otherwise) and streams x in 3 x 512 columns
into 6 PSUM banks (one accumulation group per bank, accumulated over all
16 pairs).  Drains: per-bank Z^2 partials, split ScalarE (activation
Square, single PSUM read) / VectorE (copy + fused mult+reduce).  PE HAM
warm-up dummies + an early activation-table preload keep the ramp off the
critical path.  Output: f [128, 8] f32 column partials of S6.
"""

import os
import sys

import numpy as np

for _p in ("/opt/trn_rl_repo", "/root/.axon_site/_ro/trn_rl_repo"):
    if os.path.isdir(_p) and _p not in sys.path:
        sys.path.append(_p)

import ml_dtypes

N = 4096
DX = 12288
DY = 256
NCORES = 8
KC = DX // NCORES        # 1536 x-columns per core
T = N // 128             # 32 row tiles of 128
Q = T // 2               # 16 DoubleRow pair-tiles
PCOLS = DY + KC          # combined [Y | x] row: 1792 fp8 per (pair, s)
CHUNK_PAIRS = (1, 1, 1, 1, 1, 1, 2, 2, 2, 2, 1, 1)   # DMA chunking of the 16 pairs
NWARM = 40               # PE HAM warm-up dummy matmuls (bridge until pair-0 data)

# E[fp8e4m3(x)^2] for x ~ N(0,1)  (exact; see module docstring)
C_SQ = 0.999275342216946

_PROG = None


def _build_program():
    from contextlib import ExitStack

    import concourse.bass as bass
    import concourse.tile as tile
    from concourse import bacc, mybir

    nc = bacc.Bacc(
        "TRN2",
        target_bir_lowering=False,
        debug=False,
        enable_asserts=False,
        num_devices=NCORES,
    )
    f32 = mybir.dt.float32
    bf16 = mybir.dt.bfloat16
    f8 = mybir.dt.float8e4
    DR = mybir.MatmulPerfMode.DoubleRow
    MULT = mybir.AluOpType.mult
    SQ = mybir.ActivationFunctionType.Square

    xy = nc.dram_tensor("xy", [128, Q, 2, PCOLS], f8, kind="ExternalInput").ap()
    f_out = nc.dram_tensor("f", [128, 6], f32, kind="ExternalOutput").ap()

    with tile.TileContext(nc) as tc, ExitStack() as ctx:
        data = ctx.enter_context(tc.tile_pool(name="data", bufs=1))
        scr = ctx.enter_context(tc.tile_pool(name="scr", bufs=2))
        stats = ctx.enter_context(tc.tile_pool(name="stats", bufs=1))
        zp = ctx.enter_context(tc.tile_pool(name="zp", bufs=1, space="PSUM"))

        s6 = stats.tile([128, 6], f32)
        dsrc = stats.tile([128, 128], f8)
        XY = data.tile([128, Q, 2, PCOLS], f8, name="XY")

        # All input DMAs on ONE ring in exact consumption order (strict FIFO
        # per ring -> the first chunk's descriptors drain first; a second
        # ring would round-robin SDMA packets and delay pair 0).  Pair 0 is
        # further split by columns so the first matmul (Y chunk + x cols
        # 0:512) starts after only 196KB.
        nc.sync.dma_start(XY[:, 0:1, :, 0:768], xy[:, 0:1, :, 0:768])
        nc.sync.dma_start(XY[:, 0:1, :, 768:1280], xy[:, 0:1, :, 768:1280])
        nc.sync.dma_start(XY[:, 0:1, :, 1280:PCOLS], xy[:, 0:1, :, 1280:PCOLS])
        q0 = 1
        for nq in CHUNK_PAIRS[1:]:
            nc.sync.dma_start(XY[:, q0 : q0 + nq, :, :], xy[:, q0 : q0 + nq, :, :])
            q0 += nq
        assert q0 == Q

        # warm-up while the first chunks stream: DVE memset feeds PE dummy
        # matmuls (HAM clock ramp) and ACT preloads the Square table.
        nc.vector.memset(dsrc[:], 0.0)
        tw = scr.tile([128, 1], bf16, tag="tw", name="tw")
        nc.scalar.activation(tw[:], dsrc[:, 0:1], SQ)
        zpad = zp.tile([128, 128], f32, name="zpad")
        for _ in range(NWARM):
            nc.tensor.matmul(zpad[:], lhsT=dsrc[:], rhs=dsrc[:], start=True, stop=True)

        # 6 accumulation groups g = yc*3 + xc, one PSUM bank each:
        # Z_g = Ychunk_yc^T @ Xchunk_xc accumulated over the 16 pairs.
        # Pair 0 runs xc-outer so each column piece unblocks two matmuls as
        # it lands; later pairs run yc-outer (2 LDWEIGHTS per pair).
        zb = [zp.tile([128, 512], f32, name=f"z{g}") for g in range(6)]
        for q in range(Q):
            if q == 0:
                order = [(yc, xc) for xc in range(3) for yc in range(2)]
            else:
                order = [(yc, xc) for yc in range(2) for xc in range(3)]
            for yc, xc in order:
                g = yc * 3 + xc
                nc.tensor.matmul(
                    zb[g][:],
                    lhsT=XY[:, q, :, 128 * yc : 128 * (yc + 1)],
                    rhs=XY[:, q, :, DY + 512 * xc : DY + 512 * (xc + 1)],
                    perf_mode=DR,
                    start=(q == 0),
                    stop=(q == Q - 1),
                )

        # drains: S6 partials per bank.  ACT is ~0.76us/bank (single PSUM
        # read activation Square), DVE ~1.25us/bank (copy out of PSUM, then
        # fused square+reduce from SBUF -- PSUM allows only one non-scalar
        # input), so split 4 ACT / 2 DVE, each chain in bank-stop order.
        for g in (0, 2, 4, 5):
            zsq = scr.tile([128, 512], bf16, tag="zsq", name=f"zsq{g}")
            nc.scalar.activation(zsq[:], zb[g][:], SQ, accum_out=s6[:, g : g + 1])
        for g in (1, 3):
            zcp = scr.tile([128, 512], f32, tag="zcp", name=f"zcp{g}")
            nc.vector.tensor_copy(zcp[:], zb[g][:])
            jnk = scr.tile([128, 512], bf16, tag="jnk", name=f"jnk{g}")
            nc.vector.scalar_tensor_tensor(
                out=jnk[:],
                in0=zcp[:],
                scalar=1.0,
                in1=zcp[:],
                op0=MULT,
                op1=MULT,
                accum_out=s6[:, g : g + 1],
            )

        # single contiguous output DMA on the sync ring -- its HWDGE path is
        # warm from the input stream (the idle scalar ring measured ~1.5us
        # slower).  A column split would make strided HBM writes and pay the
        # completion latency twice.
        nc.sync.dma_start(f_out, s6[:])

    nc.compile()
    return nc


def _get_program():
    global _PROG
    if _PROG is None:
        _PROG = _build_program()
    return _PROG


_LAST_RESULTS = None


def kernel(noises: np.ndarray, images: np.ndarray) -> np.ndarray:
    from concourse import bass_utils

    global _LAST_RESULTS

    nc = _get_program()

    X = np.ascontiguousarray(images, dtype=np.float32).reshape(N, -1)
    Y = np.ascontiguousarray(noises, dtype=np.float32)

    # exact host-side terms (S1..S5); f32 BLAS/einsum accumulation is ~1e-6
    # relative here, far inside the error budget.
    a = np.einsum("ij,ij->i", X, X).astype(np.float64)
    b = np.einsum("ij,ij->i", Y, Y).astype(np.float64)
    S1 = a.sum()
    S2 = b.sum()
    S3 = a @ b
    Y64 = Y.astype(np.float64)
    S4 = (Y64.T @ a) @ Y64.sum(axis=0)
    u = (X.T @ b.astype(np.float32)).astype(np.float64)
    sx = X.sum(axis=0).astype(np.float64)
    S5 = u @ sx

    # device inputs: fp8 DoubleRow-interleaved [128, Q, 2, 1792] with the Y
    # pair rows in cols 0:256 (replicated) and the core's x columns after.
    x8 = X.astype(ml_dtypes.float8_e4m3)
    y8 = Y.astype(ml_dtypes.float8_e4m3)
    y8i = np.ascontiguousarray(y8.reshape(Q, 2, 128, DY).transpose(2, 0, 1, 3))

    in_maps = []
    for c in range(NCORES):
        comb = np.empty((128, Q, 2, PCOLS), dtype=ml_dtypes.float8_e4m3)
        comb[..., :DY] = y8i
        comb[..., DY:] = (
            x8[:, c * KC : (c + 1) * KC].reshape(Q, 2, 128, KC).transpose(2, 0, 1, 3)
        )
        in_maps.append({"xy": comb})

    res = bass_utils.run_bass_kernel_spmd(nc, in_maps, core_ids=list(range(NCORES)))
    _LAST_RESULTS = res

    S6 = 0.0
    for c in range(NCORES):
        Fc = np.asarray(res.results[c]["f"], dtype=np.float64)
        S6 += Fc.sum()
    S6 /= C_SQ * C_SQ

    num = 2.0 * N * S3 + 2.0 * S1 * S2 - 4.0 * S4 - 4.0 * S5 + 4.0 * S6
    mean = num / (float(N) * N * DX * DY)
    return np.asarray(np.exp(-mean), dtype=np.float32)


# revision 27
# speedup vs baseline: 1.2491x; 1.0276x over previous
"""DiversityLoss kernel for 8 Trainium2 NeuronCores.

Reference computes:
    loss = exp(mean(-D_img * D_noise))
where D_x[i,j] = (||x_i||^2 + ||x_j||^2 - 2 (X X^T)_ij) / d_x  for X in
{images, noises}.

The pairwise matrices never need to be materialized.  With
    a_i = ||img_i||^2, b_i = ||noise_i||^2, S1 = sum a, S2 = sum b,
    S3 = a.b, S4 = (Y^T a).(Y^T 1), S5 = (X^T b).(X^T 1), S6 = ||X^T Y||_F^2
the sum over all (i,j) of D_img*D_noise * (d_x*d_y) expands exactly to
    2*N*S3 + 2*S1*S2 - 4*S4 - 4*S5 + 4*S6.

Split of work:
  - S1..S5 are O(N*d) GEMV-scale terms -> computed exactly on the host in
    fp32/fp64 (the host already reads the full fp32 inputs to quantize).
    These are also the precision-critical terms.
  - S6 = ||X^T Y||_F^2 is the only O(N*dx*dy) term -> device.  Its share of
    the final sum is ~0.025%, so fp8 precision is ample (the fp8 square
    bias is corrected by the analytic constant C_SQ^2; E[fp8e4m3(x)^2] =
    C_SQ * E[x^2] for x ~ N(0,1)).

Sharding: the feature (column) axis of the flattened images is split across
the 8 cores (1536 columns each); noises Y is replicated.  S6 splits over
column chunks with no cross-core reduction.

Per-core device program: one input tensor in DoubleRow-interleaved layout
[128, 16 pairs, 2, 1792] fp8 where cols 0:256 of each pair-row are the Y
pair rows and cols 256:1792 the x pair rows.  The DMA streams pair chunks
in consumption order; for each pair the matmul makes the 128-col Y chunk
stationary (2 LDWEIGHTS per pair instead of 12 -- LDWEIGHTS at 1 row/cycle
@1.2GHz is the PE bottleneck otherwise) and streams x in 3 x 512 columns
into 6 PSUM banks (one accumulation group per bank, accumulated over all
16 pairs).  Drains: per-bank Z^2 partials, split ScalarE (activation
Square, single PSUM read) / VectorE (copy + fused mult+reduce).  PE HAM
warm-up dummies + an early activation-table preload keep the ramp off the
critical path.  Output: f [128, 8] f32 column partials of S6.
"""

import os
import sys

import numpy as np

for _p in ("/opt/trn_rl_repo", "/root/.axon_site/_ro/trn_rl_repo"):
    if os.path.isdir(_p) and _p not in sys.path:
        sys.path.append(_p)

import ml_dtypes

N = 4096
DX = 12288
DY = 256
NCORES = 8
KC = DX // NCORES        # 1536 x-columns per core
T = N // 128             # 32 row tiles of 128
Q = T // 2               # 16 DoubleRow pair-tiles
PCOLS = DY + KC          # combined [Y | x] row: 1792 fp8 per (pair, s)
CHUNK_PAIRS = (1, 1, 1, 1, 1, 1, 1, 1, 2, 2, 2, 1, 1)   # DMA chunking of the 16 pairs
NWARM = 40               # PE HAM warm-up dummy matmuls (bridge until pair-0 data)

# E[fp8e4m3(x)^2] for x ~ N(0,1)  (exact; see module docstring)
C_SQ = 0.999275342216946

_PROG = None


def _build_program():
    from contextlib import ExitStack

    import concourse.bass as bass
    import concourse.tile as tile
    from concourse import bacc, mybir

    nc = bacc.Bacc(
        "TRN2",
        target_bir_lowering=False,
        debug=False,
        enable_asserts=False,
        num_devices=NCORES,
    )
    f32 = mybir.dt.float32
    bf16 = mybir.dt.bfloat16
    f8 = mybir.dt.float8e4
    DR = mybir.MatmulPerfMode.DoubleRow
    MULT = mybir.AluOpType.mult
    SQ = mybir.ActivationFunctionType.Square

    xy = nc.dram_tensor("xy", [128, Q, 2, PCOLS], f8, kind="ExternalInput").ap()
    f_out = nc.dram_tensor("f", [128, 6], f32, kind="ExternalOutput").ap()

    with tile.TileContext(nc) as tc, ExitStack() as ctx:
        data = ctx.enter_context(tc.tile_pool(name="data", bufs=1))
        scr = ctx.enter_context(tc.tile_pool(name="scr", bufs=2))
        stats = ctx.enter_context(tc.tile_pool(name="stats", bufs=1))
        zp = ctx.enter_context(tc.tile_pool(name="zp", bufs=1, space="PSUM"))

        s6 = stats.tile([128, 6], f32)
        dsrc = stats.tile([128, 128], f8)
        XY = data.tile([128, Q, 2, PCOLS], f8, name="XY")

        # All input DMAs on ONE ring in exact consumption order (strict FIFO
        # per ring -> the first chunk's descriptors drain first; a second
        # ring would round-robin SDMA packets and delay pair 0).  Pair 0 is
        # further split by columns so the first matmul (Y chunk + x cols
        # 0:512) starts after only 196KB.
        # pair-1's chunk ships between pair-0's column pieces: its semaphore
        # then lands right as the PE drains pair-0's first two matmuls,
        # instead of 1us after pair-0 completes.
        nc.sync.dma_start(XY[:, 0:1, :, 0:768], xy[:, 0:1, :, 0:768])
        nc.sync.dma_start(XY[:, 1:2, :, :], xy[:, 1:2, :, :])
        nc.sync.dma_start(XY[:, 0:1, :, 768:1280], xy[:, 0:1, :, 768:1280])
        nc.sync.dma_start(XY[:, 0:1, :, 1280:PCOLS], xy[:, 0:1, :, 1280:PCOLS])
        q0 = 2
        for nq in CHUNK_PAIRS[2:]:
            nc.sync.dma_start(XY[:, q0 : q0 + nq, :, :], xy[:, q0 : q0 + nq, :, :])
            q0 += nq
        assert q0 == Q

        # warm-up while the first chunks stream: DVE memset feeds PE dummy
        # matmuls (HAM clock ramp) and ACT preloads the Square table.
        nc.vector.memset(dsrc[:], 0.0)
        tw = scr.tile([128, 1], bf16, tag="tw", name="tw")
        nc.scalar.activation(tw[:], dsrc[:, 0:1], SQ)
        zpad = zp.tile([128, 128], f32, name="zpad")
        for _ in range(NWARM):
            nc.tensor.matmul(zpad[:], lhsT=dsrc[:], rhs=dsrc[:], start=True, stop=True)

        # 6 accumulation groups g = yc*3 + xc, one PSUM bank each:
        # Z_g = Ychunk_yc^T @ Xchunk_xc accumulated over the 16 pairs.
        # Consumption order matches the DMA stream: pair-0 piece a, all of
        # pair 1, pair-0 pieces b/c, then pair-outer (2 LDWEIGHTS per pair).
        # start= is per-group first emission (banks are group-private, so
        # accumulation order within a group is free).
        zb = [zp.tile([128, 512], f32, name=f"z{g}") for g in range(6)]
        started = set()

        def mm(q, yc, xc):
            g = yc * 3 + xc
            nc.tensor.matmul(
                zb[g][:],
                lhsT=XY[:, q, :, 128 * yc : 128 * (yc + 1)],
                rhs=XY[:, q, :, DY + 512 * xc : DY + 512 * (xc + 1)],
                perf_mode=DR,
                start=(g not in started),
                stop=(q == Q - 1),
            )
            started.add(g)

        mm(0, 0, 0)
        mm(0, 1, 0)
        for yc in range(2):
            for xc in range(3):
                mm(1, yc, xc)
        for xc in (1, 2):
            mm(0, 0, xc)
            mm(0, 1, xc)
        for q in range(2, Q):
            for yc in range(2):
                for xc in range(3):
                    mm(q, yc, xc)

        # drains: S6 partials per bank.  ACT is ~0.76us/bank (single PSUM
        # read activation Square), DVE ~1.25us/bank (copy out of PSUM, then
        # fused square+reduce from SBUF -- PSUM allows only one non-scalar
        # input), so split 4 ACT / 2 DVE, each chain in bank-stop order.
        for g in (0, 2, 4, 5):
            zsq = scr.tile([128, 512], bf16, tag="zsq", name=f"zsq{g}")
            nc.scalar.activation(zsq[:], zb[g][:], SQ, accum_out=s6[:, g : g + 1])
        for g in (1, 3):
            zcp = scr.tile([128, 512], f32, tag="zcp", name=f"zcp{g}")
            nc.vector.tensor_copy(zcp[:], zb[g][:])
            jnk = scr.tile([128, 512], bf16, tag="jnk", name=f"jnk{g}")
            nc.vector.scalar_tensor_tensor(
                out=jnk[:],
                in0=zcp[:],
                scalar=1.0,
                in1=zcp[:],
                op0=MULT,
                op1=MULT,
                accum_out=s6[:, g : g + 1],
            )

        # single contiguous output DMA on the sync ring -- its HWDGE path is
        # warm from the input stream (the idle scalar ring measured ~1.5us
        # slower).  A column split would make strided HBM writes and pay the
        # completion latency twice.
        nc.sync.dma_start(f_out, s6[:])

    nc.compile()
    return nc


def _get_program():
    global _PROG
    if _PROG is None:
        _PROG = _build_program()
    return _PROG


_LAST_RESULTS = None


def kernel(noises: np.ndarray, images: np.ndarray) -> np.ndarray:
    from concourse import bass_utils

    global _LAST_RESULTS

    nc = _get_program()

    X = np.ascontiguousarray(images, dtype=np.float32).reshape(N, -1)
    Y = np.ascontiguousarray(noises, dtype=np.float32)

    # exact host-side terms (S1..S5); f32 BLAS/einsum accumulation is ~1e-6
    # relative here, far inside the error budget.
    a = np.einsum("ij,ij->i", X, X).astype(np.float64)
    b = np.einsum("ij,ij->i", Y, Y).astype(np.float64)
    S1 = a.sum()
    S2 = b.sum()
    S3 = a @ b
    Y64 = Y.astype(np.float64)
    S4 = (Y64.T @ a) @ Y64.sum(axis=0)
    u = (X.T @ b.astype(np.float32)).astype(np.float64)
    sx = X.sum(axis=0).astype(np.float64)
    S5 = u @ sx

    # device inputs: fp8 DoubleRow-interleaved [128, Q, 2, 1792] with the Y
    # pair rows in cols 0:256 (replicated) and the core's x columns after.
    x8 = X.astype(ml_dtypes.float8_e4m3)
    y8 = Y.astype(ml_dtypes.float8_e4m3)
    y8i = np.ascontiguousarray(y8.reshape(Q, 2, 128, DY).transpose(2, 0, 1, 3))

    in_maps = []
    for c in range(NCORES):
        comb = np.empty((128, Q, 2, PCOLS), dtype=ml_dtypes.float8_e4m3)
        comb[..., :DY] = y8i
        comb[..., DY:] = (
            x8[:, c * KC : (c + 1) * KC].reshape(Q, 2, 128, KC).transpose(2, 0, 1, 3)
        )
        in_maps.append({"xy": comb})

    res = bass_utils.run_bass_kernel_spmd(nc, in_maps, core_ids=list(range(NCORES)))
    _LAST_RESULTS = res

    S6 = 0.0
    for c in range(NCORES):
        Fc = np.asarray(res.results[c]["f"], dtype=np.float64)
        S6 += Fc.sum()
    S6 /= C_SQ * C_SQ

    num = 2.0 * N * S3 + 2.0 * S1 * S2 - 4.0 * S4 - 4.0 * S5 + 4.0 * S6
    mean = num / (float(N) * N * DX * DY)
    return np.asarray(np.exp(-mean), dtype=np.float32)
